# revision 7
# baseline (speedup 1.0000x reference)
"""DependencyBertSelfAttention Trainium2 kernel.

Sharding: batch B=8 -> one batch element per NeuronCore (8 cores, SPMD).
Per core (full T=1024, C=768, H=12 heads, D=64):

  Stage A: QKV projections as f32r matmuls.
    Q^T, K^T computed in [C, T] layout (partition = channel), with bias and
    the 1/sqrt(D) score scale folded into the Q eviction.
    V computed in natural [T, C] layout, evicted (bf16) into an interleaved
    v_aug layout: per head 65 columns = [V_h (64) | ones], so the PV matmul
    also produces the softmax denominator.
  Stage B (per head): scores S^T[s,t] = K_h^T.T @ Q_h^T (f32r, k=64);
    branch 1: exp(S) -> pO (bf16); branch 2: S *= depT (DVE, in psum),
    exp -> pD (bf16). PV: out[t, 65] accumulated over s-blocks with
    lhsT = P^T blocks, rhs = v_aug head slice; col 64 = sum(P) denominator.
    Normalization by reciprocal(denominator) during psum eviction.
  Stage C (per t-block): tanh, gate z = sum(tanh(osa)*WgO + tanh(dsa)*WgD),
    g = 1/(1+exp(-(z+bg))), out = g*osa + (1-g)*dsa.

No inter-core communication: each core's batch element is independent.
"""
import sys

sys.path.insert(0, "/opt/trn_rl_repo")

import numpy as np
import ml_dtypes
from contextlib import ExitStack

import concourse.bass as bass
import concourse.tile as tile
from concourse import bacc, mybir

B, T, C, H, D = 8, 1024, 768, 12, 64
CB = C // 128   # 6 channel partition-blocks
SB = T // 128   # 8 s/t blocks
NCORES = 8

F32 = mybir.dt.float32
F32R = mybir.dt.float32r
BF16 = mybir.dt.bfloat16
AF = mybir.ActivationFunctionType
ALU = mybir.AluOpType

IN_PLACE_SD = False  # dep-multiply writes back into the scores psum tile


def build_nc(debug=False):
    nc = bacc.Bacc("TRN2", target_bir_lowering=False, debug=False,
                   num_devices=NCORES)

    xT_d = nc.dram_tensor("xT", [C, T], F32, kind="ExternalInput").ap()
    wq_d = nc.dram_tensor("wq", [C, C], F32, kind="ExternalInput").ap()
    wk_d = nc.dram_tensor("wk", [C, C], F32, kind="ExternalInput").ap()
    wv_d = nc.dram_tensor("wv", [C, C], F32, kind="ExternalInput").ap()
    bqs_d = nc.dram_tensor("bqs", [C, 1], F32, kind="ExternalInput").ap()
    bk_d = nc.dram_tensor("bk", [C, 1], F32, kind="ExternalInput").ap()
    bv_d = nc.dram_tensor("bv", [C], F32, kind="ExternalInput").ap()
    dep_d = nc.dram_tensor("dep", [T, T], BF16, kind="ExternalInput").ap()
    wg_d = nc.dram_tensor("wg", [2 * C], F32, kind="ExternalInput").ap()
    nbg_d = nc.dram_tensor("nbg", [1], F32, kind="ExternalInput").ap()
    out_d = nc.dram_tensor("out", [T, C], F32, kind="ExternalOutput").ap()
    if debug:
        dbg = {
            "dbg_qT0": nc.dram_tensor("dbg_qT0", [128, T], F32, kind="ExternalOutput").ap(),
            "dbg_kT0": nc.dram_tensor("dbg_kT0", [128, T], F32, kind="ExternalOutput").ap(),
            "dbg_vaug0": nc.dram_tensor("dbg_vaug0", [128, H * 65], BF16, kind="ExternalOutput").ap(),
            "dbg_pO": nc.dram_tensor("dbg_pO", [128, T], BF16, kind="ExternalOutput").ap(),
            "dbg_pD": nc.dram_tensor("dbg_pD", [128, T], BF16, kind="ExternalOutput").ap(),
            "dbg_osa0": nc.dram_tensor("dbg_osa0", [128, C], F32, kind="ExternalOutput").ap(),
            "dbg_dsa0": nc.dram_tensor("dbg_dsa0", [128, C], F32, kind="ExternalOutput").ap(),
        }

    def bcast(src_ap, n_free):
        return bass.AP(tensor=src_ap.tensor, offset=src_ap.offset,
                       ap=[[0, 128], [1, n_free]])

    with tile.TileContext(nc) as tc, ExitStack() as ctx:
        persist = ctx.enter_context(tc.tile_pool(name="persist", bufs=1))

        # ---- persistent tiles
        qT = [persist.tile([128, T], F32R, tag=f"qT{i}", name=f"qT{i}") for i in range(CB)]
        kT = [persist.tile([128, T], F32R, tag=f"kT{i}", name=f"kT{i}") for i in range(CB)]
        vaug = [persist.tile([128, H * 65], BF16, tag=f"vaug{i}", name=f"vaug{i}") for i in range(SB)]
        dep_t = [persist.tile([128, T], BF16, tag=f"dep{i}", name=f"dep{i}") for i in range(SB)]
        wgb = persist.tile([128, 2 * C], F32, tag="wgb", name="wgb")
        nbg_t = persist.tile([128, 1], F32, tag="nbg", name="nbg")
        bq_t = [persist.tile([128, 1], F32, tag=f"bq{i}", name=f"bq{i}") for i in range(CB)]
        bk_t = [persist.tile([128, 1], F32, tag=f"bk{i}", name=f"bk{i}") for i in range(CB)]

        for i in range(SB):
            nc.sync.dma_start(dep_t[i][:], dep_d[i * 128:(i + 1) * 128, :])
        nc.sync.dma_start(wgb[:], bcast(wg_d, 2 * C))
        nc.sync.dma_start(nbg_t[:], bcast(nbg_d, 1))
        for i in range(CB):
            nc.sync.dma_start(bq_t[i][:], bqs_d[i * 128:(i + 1) * 128, :])
            nc.sync.dma_start(bk_t[i][:], bk_d[i * 128:(i + 1) * 128, :])

        # ================= Stage A: projections =================
        with ExitStack() as actx:
            sa = actx.enter_context(tc.tile_pool(name="stageA", bufs=1))
            psA = actx.enter_context(tc.tile_pool(name="psA", bufs=3, space="PSUM"))

            xT = [sa.tile([128, T], F32R, tag=f"xT{i}", name=f"xT{i}") for i in range(CB)]
            for i in range(CB):
                nc.sync.dma_start(xT[i][:], xT_d[i * 128:(i + 1) * 128, :].bitcast(F32R))
            wts = {}
            for wname, w_d in (("q", wq_d), ("k", wk_d), ("v", wv_d)):
                wts[wname] = [sa.tile([128, C], F32R, tag=f"w{wname}{i}", name=f"w{wname}{i}")
                              for i in range(CB)]
                for i in range(CB):
                    nc.sync.dma_start(wts[wname][i][:],
                                      w_d[i * 128:(i + 1) * 128, :].bitcast(F32R))
            bvb = sa.tile([128, C], F32, tag="bvb", name="bvb")
            nc.sync.dma_start(bvb[:], bcast(bv_d, C))

            # Q^T and K^T: out[c' part, t free]
            for dst, w, bias, scale in ((qT, wts["q"], bq_t, 0.125),
                                        (kT, wts["k"], bk_t, 1.0)):
                for cb in range(CB):
                    for tch in range(2):
                        ps = psA.tile([128, 512], F32, tag="psA", name="psA")
                        for kb in range(CB):
                            nc.tensor.matmul(
                                ps[:],
                                w[kb][:, cb * 128:(cb + 1) * 128],
                                xT[kb][:, tch * 512:(tch + 1) * 512],
                                start=(kb == 0), stop=(kb == CB - 1))
                        nc.scalar.activation(
                            dst[cb][:, tch * 512:(tch + 1) * 512], ps[:],
                            AF.Identity, bias=bias[cb][:], scale=scale)

            # V natural: out[s part, c' free] -> v_aug interleaved + ones
            for sb in range(SB):
                va3 = vaug[sb][:].rearrange("p (h d) -> p h d", d=65)
                for ch, (n0, nw) in enumerate(((0, 512), (512, 256))):
                    ps = psA.tile([128, 512], F32, tag="psA", name="psA")
                    for kb in range(CB):
                        nc.tensor.matmul(
                            ps[:, 0:nw],
                            xT[kb][:, sb * 128:(sb + 1) * 128],
                            wts["v"][kb][:, n0:n0 + nw],
                            start=(kb == 0), stop=(kb == CB - 1))
                    nh = nw // 64
                    h0 = n0 // 64
                    ps3 = ps[:, 0:nw].rearrange("p (h d) -> p h d", d=64)
                    bv3 = bvb[:, n0:n0 + nw].rearrange("p (h d) -> p h d", d=64)
                    nc.vector.tensor_add(va3[:, h0:h0 + nh, 0:64], ps3, bv3)
                nc.vector.memset(va3[:, :, 64:65], 1.0)

        # ================= Stage B: attention per head =================
        with ExitStack() as bctx:
            pb = bctx.enter_context(tc.tile_pool(name="pP", bufs=3))
            sdp = bctx.enter_context(tc.tile_pool(name="sdp", bufs=2))
            rp = bctx.enter_context(tc.tile_pool(name="rp", bufs=6))
            psS = bctx.enter_context(tc.tile_pool(name="psS", bufs=2, space="PSUM"))
            psPV = bctx.enter_context(tc.tile_pool(name="psPV", bufs=1, space="PSUM"))
            cp = bctx.enter_context(tc.tile_pool(name="stageC", bufs=2))
            zp = bctx.enter_context(tc.tile_pool(name="zP", bufs=3))
            od = bctx.enter_context(tc.tile_pool(name="odP", bufs=1))
            osa = [od.tile([128, C], F32, tag=f"osa{i}", name=f"osa{i}") for i in range(SB)]
            dsa = [od.tile([128, C], F32, tag=f"dsa{i}", name=f"dsa{i}") for i in range(SB)]

            for h in range(H):
                hb, hoff = h // 2, (h % 2) * 64
                pOs, pDs = [], []
                for sb in range(SB):
                    ps = psS.tile([128, T], F32, tag="psS", name="psS")
                    for tch in range(2):
                        nc.tensor.matmul(
                            ps[:, tch * 512:(tch + 1) * 512],
                            kT[hb][hoff:hoff + 64, sb * 128:(sb + 1) * 128],
                            qT[hb][hoff:hoff + 64, tch * 512:(tch + 1) * 512],
                            start=True, stop=True)
                    pO = pb.tile([128, T], BF16, tag="pO", name="pO")
                    nc.scalar.activation(pO[:], ps[:], AF.Exp)
                    if IN_PLACE_SD:
                        nc.vector.tensor_mul(ps[:], ps[:], dep_t[sb][:])
                        sd_src = ps
                    else:
                        sd = sdp.tile([128, T], F32, tag="sd", name="sd")
                        nc.vector.tensor_mul(sd[:], ps[:], dep_t[sb][:])
                        sd_src = sd
                    pD = pb.tile([128, T], BF16, tag="pD", name="pD")
                    nc.scalar.activation(pD[:], sd_src[:], AF.Exp)
                    pOs.append(pO)
                    pDs.append(pD)
                    if debug and h == 0 and sb == 0:
                        nc.sync.dma_start(dbg["dbg_pO"][:], pO[:])
                        nc.sync.dma_start(dbg["dbg_pD"][:], pD[:])

                    # PV for this s-block into the 4 tb-pair psum tiles
                    for tbp in range(4):
                        if sb == 0:
                            ppv = psPV.tile([128, 260], F32, tag=f"ppv{tbp}", name=f"ppv{tbp}")
                            if tbp == 0:
                                ppvs = []
                            ppvs.append(ppv)
                        ppv = ppvs[tbp]
                        for half in range(2):
                            tb = tbp * 2 + half
                            base = half * 130
                            # start=True clears has_written for the WHOLE
                            # bank, so only the very first matmul into this
                            # psum tile may set it; the other column-ranges'
                            # first writes overwrite via cleared bits.
                            nc.tensor.matmul(
                                ppv[:, base:base + 65],
                                pO[:, tb * 128:(tb + 1) * 128],
                                vaug[sb][:, h * 65:h * 65 + 65],
                                start=(sb == 0 and half == 0),
                                stop=(sb == SB - 1))
                            nc.tensor.matmul(
                                ppv[:, base + 65:base + 130],
                                pD[:, tb * 128:(tb + 1) * 128],
                                vaug[sb][:, h * 65:h * 65 + 65],
                                start=False, stop=(sb == SB - 1))

                # normalize + evict
                for tbp in range(4):
                    ppv = ppvs[tbp]
                    rec = rp.tile([128, 4], F32, tag="rec", name="rec")
                    den3 = ppv[:].rearrange("p (g d) -> p g d", d=65)[:, :, 64:65]
                    nc.vector.reciprocal(
                        rec[:].rearrange("p (g d) -> p g d", d=1), den3)
                    for half in range(2):
                        tb = tbp * 2 + half
                        base = half * 130
                        nc.vector.tensor_scalar(
                            osa[tb][:, h * 64:(h + 1) * 64],
                            ppv[:, base:base + 64],
                            rec[:, 2 * half:2 * half + 1], None, ALU.mult)
                        nc.vector.tensor_scalar(
                            dsa[tb][:, h * 64:(h + 1) * 64],
                            ppv[:, base + 65:base + 129],
                            rec[:, 2 * half + 1:2 * half + 2], None, ALU.mult)

            if debug:
                nc.sync.dma_start(dbg["dbg_qT0"][:], qT[0][:].bitcast(F32))
                nc.sync.dma_start(dbg["dbg_kT0"][:], kT[0][:].bitcast(F32))
                nc.sync.dma_start(dbg["dbg_vaug0"][:], vaug[0][:])
                nc.sync.dma_start(dbg["dbg_osa0"][:], osa[0][:])
                nc.sync.dma_start(dbg["dbg_dsa0"][:], dsa[0][:])
            # ================= Stage C: gate + blend =================
            for tb in range(SB):
                tosa = cp.tile([128, C], F32, tag="tosa", name="tosa")
                tdsa = cp.tile([128, C], F32, tag="tdsa", name="tdsa")
                nc.scalar.activation(tosa[:], osa[tb][:], AF.Tanh)
                nc.scalar.activation(tdsa[:], dsa[tb][:], AF.Tanh)
                scr = cp.tile([128, C], F32, tag="scr", name="scr")
                zO = zp.tile([128, 1], F32, tag="zO", name="zO")
                z = zp.tile([128, 1], F32, tag="z", name="z")
                nc.vector.scalar_tensor_tensor(
                    out=scr[:], in0=tosa[:], scalar=0.0, in1=wgb[:, 0:C],
                    op0=ALU.bypass, op1=ALU.mult, accum_out=zO[:])
                scr2 = cp.tile([128, C], F32, tag="scr2", name="scr2")
                zD = zp.tile([128, 1], F32, tag="zD", name="zD")
                nc.vector.scalar_tensor_tensor(
                    out=scr2[:], in0=tdsa[:], scalar=0.0, in1=wgb[:, C:2 * C],
                    op0=ALU.bypass, op1=ALU.mult, accum_out=zD[:])
                nc.vector.tensor_add(z[:], zO[:], zD[:])
                e = zp.tile([128, 1], F32, tag="e", name="e")
                # e = exp(-(z + bg)) = exp(-z + nbg)
                nc.scalar.activation(e[:], z[:], AF.Exp, bias=nbg_t[:], scale=-1.0)
                g = zp.tile([128, 1], F32, tag="g", name="g")
                nc.vector.tensor_scalar_add(e[:], e[:], 1.0)
                nc.vector.reciprocal(g[:], e[:])
                diff = cp.tile([128, C], F32, tag="diff", name="diff")
                nc.vector.tensor_sub(diff[:], osa[tb][:], dsa[tb][:])
                outt = cp.tile([128, C], F32, tag="outt", name="outt")
                nc.vector.scalar_tensor_tensor(
                    out=outt[:], in0=diff[:], scalar=g[:], in1=dsa[tb][:],
                    op0=ALU.mult, op1=ALU.add)
                nc.sync.dma_start(out_d[tb * 128:(tb + 1) * 128, :], outt[:])

    nc.finalize()
    return nc


_CACHE = {}


def _prep_in_maps(hidden_states, dependency_matrix, Wq, bq, Wk, bk, Wv, bv, Wg, bg):
    hs = np.asarray(hidden_states, dtype=np.float32)
    dep = np.asarray(dependency_matrix, dtype=np.float32)
    shared = {
        "wq": np.ascontiguousarray(np.asarray(Wq, np.float32).T),
        "wk": np.ascontiguousarray(np.asarray(Wk, np.float32).T),
        "wv": np.ascontiguousarray(np.asarray(Wv, np.float32).T),
        "bqs": (np.asarray(bq, np.float32) * 0.125).reshape(C, 1),
        "bk": np.asarray(bk, np.float32).reshape(C, 1),
        "bv": np.ascontiguousarray(np.asarray(bv, np.float32).reshape(C)),
        "wg": np.ascontiguousarray(np.asarray(Wg, np.float32).reshape(2 * C)),
        "nbg": (-np.asarray(bg, np.float32)).reshape(1),
    }
    in_maps = []
    for b in range(B):
        m = dict(shared)
        m["xT"] = np.ascontiguousarray(hs[b].T)
        m["dep"] = np.ascontiguousarray(dep[b].T).astype(ml_dtypes.bfloat16)
        in_maps.append(m)
    return in_maps


def kernel(**inputs):
    from concourse.bass_utils import run_bass_kernel_spmd
    if "nc" not in _CACHE:
        _CACHE["nc"] = build_nc()
    nc = _CACHE["nc"]
    in_maps = _prep_in_maps(**inputs)
    res = run_bass_kernel_spmd(nc, in_maps, core_ids=list(range(NCORES)))
    out = np.stack([res.results[i]["out"] for i in range(NCORES)], axis=0)
    return out.astype(np.float32)


# revision 21
# speedup vs baseline: 248.4094x; 248.4094x over previous
"""DependencyBertSelfAttention Trainium2 kernel.

Sharding: batch B=8 -> one batch element per NeuronCore (8 cores, SPMD).
Per core (full T=1024, C=768, H=12 heads, D=64):

  Stage A: QKV projections as f32r matmuls.
    Q^T, K^T computed in [C, T] layout (partition = channel), with bias and
    the 1/sqrt(D) score scale folded into the Q eviction (DVE).
    V computed in natural [T, C] layout, evicted (bf16) into an interleaved
    v_aug layout: per head 65 columns = [V_h (64) | ones], so the PV matmul
    also produces the softmax denominator.
    Ordered so head 0/1 inputs (qT[0], kT[0], vaug) finish first and the
    attention stage overlaps the tail of the projections.
  Stage B (per head): scores S^T[s,t] = K_h^T.T @ Q_h^T (f32r, k=64);
    branch 1: exp(S) -> pO (bf16); branch 2: S *= depT (DVE, in psum),
    exp -> pD (bf16). PV: out[t, 65] accumulated over s-blocks with
    lhsT = P^T blocks, rhs = v_aug head slice; col 64 = sum(P) denominator.
    start=True only on the first matmul into each psum bank (the start flag
    clears has_written for the whole bank).
    Normalization by reciprocal(denominator) during the batched psum
    eviction into the merged od[tb] = [osa | dsa] tile.
  Stage C (per t-block): one tanh over [osa|dsa], gate
    z = sum(tanh(od) * Wg) in one scalar_tensor_tensor with accumulate,
    g = 1/(1+exp(-(z+bg))), out = g*osa + (1-g)*dsa.

No inter-core communication: each core's batch element is independent.
"""
import sys

sys.path.insert(0, "/opt/trn_rl_repo")

import numpy as np
import ml_dtypes
from contextlib import ExitStack

import concourse.bass as bass
import concourse.tile as tile
from concourse import bacc, mybir

B, T, C, H, D = 8, 1024, 768, 12, 64
CB = C // 128   # 6 channel partition-blocks
SB = T // 128   # 8 s/t blocks
NCORES = 8

F32 = mybir.dt.float32
F32R = mybir.dt.float32r
BF16 = mybir.dt.bfloat16
AF = mybir.ActivationFunctionType
ALU = mybir.AluOpType

IN_PLACE_SD = False    # dep-multiply writes back into the scores psum tile
BCAST_NORM = True     # batched PV eviction via step-0 free-dim broadcast


def build_nc(debug=False):
    nc = bacc.Bacc("TRN2", target_bir_lowering=False, debug=False,
                   num_devices=NCORES)

    xT_d = nc.dram_tensor("xT", [C, T], F32, kind="ExternalInput").ap()
    wq_d = nc.dram_tensor("wq", [C, C], F32, kind="ExternalInput").ap()
    wk_d = nc.dram_tensor("wk", [C, C], F32, kind="ExternalInput").ap()
    wv_d = nc.dram_tensor("wv", [C, C], F32, kind="ExternalInput").ap()
    bqs_d = nc.dram_tensor("bqs", [C, 1], F32, kind="ExternalInput").ap()
    bk_d = nc.dram_tensor("bk", [C, 1], F32, kind="ExternalInput").ap()
    bv_d = nc.dram_tensor("bv", [C], F32, kind="ExternalInput").ap()
    dep_d = nc.dram_tensor("dep", [T, T], BF16, kind="ExternalInput").ap()
    wg_d = nc.dram_tensor("wg", [2 * C], F32, kind="ExternalInput").ap()
    nbg_d = nc.dram_tensor("nbg", [1], F32, kind="ExternalInput").ap()
    out_d = nc.dram_tensor("out", [T, C], F32, kind="ExternalOutput").ap()
    if debug:
        dbg = {
            "dbg_qT0": nc.dram_tensor("dbg_qT0", [128, T], F32, kind="ExternalOutput").ap(),
            "dbg_kT0": nc.dram_tensor("dbg_kT0", [128, T], F32, kind="ExternalOutput").ap(),
            "dbg_vaug0": nc.dram_tensor("dbg_vaug0", [128, H * 65], BF16, kind="ExternalOutput").ap(),
            "dbg_pO": nc.dram_tensor("dbg_pO", [128, T], BF16, kind="ExternalOutput").ap(),
            "dbg_pD": nc.dram_tensor("dbg_pD", [128, T], BF16, kind="ExternalOutput").ap(),
            "dbg_osa0": nc.dram_tensor("dbg_osa0", [128, C], F32, kind="ExternalOutput").ap(),
            "dbg_dsa0": nc.dram_tensor("dbg_dsa0", [128, C], F32, kind="ExternalOutput").ap(),
        }

    def bcast(src_ap, n_free):
        return bass.AP(tensor=src_ap.tensor, offset=src_ap.offset,
                       ap=[[0, 128], [1, n_free]])

    with tile.TileContext(nc, pool_alloc_mode="queue") as tc, ExitStack() as ctx:
        persist = ctx.enter_context(tc.tile_pool(name="persist", bufs=1))
        psS = ctx.enter_context(tc.tile_pool(name="psS", bufs=2, space="PSUM"))

        # ---- persistent tiles
        qT = [persist.tile([128, T], F32R, tag=f"qT{i}", name=f"qT{i}") for i in range(CB)]
        kT = [persist.tile([128, T], F32R, tag=f"kT{i}", name=f"kT{i}") for i in range(CB)]
        vaug = [persist.tile([128, H * 65], BF16, tag=f"vaug{i}", name=f"vaug{i}") for i in range(SB)]
        dep_t = [persist.tile([128, T], BF16, tag=f"dep{i}", name=f"dep{i}") for i in range(SB)]
        wgb = persist.tile([128, 2 * C], F32, tag="wgb", name="wgb")
        nbg_t = persist.tile([128, 1], F32, tag="nbg", name="nbg")
        bq_t = [persist.tile([128, 1], F32, tag=f"bq{i}", name=f"bq{i}") for i in range(CB)]
        bk_t = [persist.tile([128, 1], F32, tag=f"bk{i}", name=f"bk{i}") for i in range(CB)]


        # ================= Stage A: projections =================
        with ExitStack() as actx:
            sa = actx.enter_context(tc.tile_pool(name="stageA", bufs=1))
            psA = actx.enter_context(tc.tile_pool(name="psA", bufs=5, space="PSUM"))

            xT = [sa.tile([128, T], F32R, tag=f"xT{i}", name=f"xT{i}") for i in range(CB)]
            wts = {}
            for wname, w_d in (("q", wq_d), ("k", wk_d), ("v", wv_d)):
                wts[wname] = [sa.tile([128, C], F32R, tag=f"w{wname}{i}", name=f"w{wname}{i}")
                              for i in range(CB)]
            # xT on sync queues, weights on gpsimd queues, in parallel;
            # head-0 critical inputs (xT, wq, wk) first.
            for i in range(CB):
                nc.sync.dma_start(xT[i][:], xT_d[i * 128:(i + 1) * 128, :].bitcast(F32R))
            for wname, w_d in (("q", wq_d), ("k", wk_d)):
                for i in range(CB):
                    nc.gpsimd.dma_start(wts[wname][i][:],
                                        w_d[i * 128:(i + 1) * 128, :].bitcast(F32R))
            for i in range(CB):
                nc.sync.dma_start(bq_t[i][:], bqs_d[i * 128:(i + 1) * 128, :])
                nc.sync.dma_start(bk_t[i][:], bk_d[i * 128:(i + 1) * 128, :])
            for i in range(CB):
                nc.gpsimd.dma_start(wts["v"][i][:],
                                    wv_d[i * 128:(i + 1) * 128, :].bitcast(F32R))
            bvb = sa.tile([128, C], F32, tag="bvb", name="bvb")
            nc.gpsimd.dma_start(bvb[:], bcast(bv_d, C))
            for i in range(SB):
                nc.sync.dma_start(dep_t[i][:], dep_d[i * 128:(i + 1) * 128, :])
            nc.sync.dma_start(wgb[:], bcast(wg_d, 2 * C))
            nc.sync.dma_start(nbg_t[:], bcast(nbg_d, 1))

            def proj_qk(cb):
                # Q^T and K^T chunk: out[c' part, t free]; evict on DVE:
                # (psum + bias) * scale, rounded to f32r.
                for dst, w, bias, scale in ((qT, wts["q"], bq_t, 0.125),
                                            (kT, wts["k"], bk_t, 1.0)):
                    for tch in range(2):
                        ps = psA.tile([128, 512], F32, tag="psA", name="psA")
                        for kb in range(CB):
                            nc.tensor.matmul(
                                ps[:],
                                w[kb][:, cb * 128:(cb + 1) * 128],
                                xT[kb][:, tch * 512:(tch + 1) * 512],
                                start=(kb == 0), stop=(kb == CB - 1))
                        nc.scalar.activation(
                            dst[cb][:, tch * 512:(tch + 1) * 512], ps[:],
                            AF.Identity, bias=bias[cb][:], scale=scale)

            def proj_v(sb):
                # V natural: out[s part, c' free] -> v_aug interleaved + ones
                va3 = vaug[sb][:].rearrange("p (h d) -> p h d", d=65)
                for ch, (n0, nw) in enumerate(((0, 512), (512, 256))):
                    ps = psA.tile([128, 512], F32, tag="psA", name="psA")
                    for kb in range(CB):
                        nc.tensor.matmul(
                            ps[:, 0:nw],
                            xT[kb][:, sb * 128:(sb + 1) * 128],
                            wts["v"][kb][:, n0:n0 + nw],
                            start=(kb == 0), stop=(kb == CB - 1))
                    nh = nw // 64
                    h0 = n0 // 64
                    ps3 = ps[:, 0:nw].rearrange("p (h d) -> p h d", d=64)
                    bv3 = bvb[:, n0:n0 + nw].rearrange("p (h d) -> p h d", d=64)
                    nc.vector.tensor_add(va3[:, h0:h0 + nh, 0:64], ps3, bv3)
                nc.vector.memset(va3[:, :, 64:65], 1.0)

            # head 0/1 inputs first so attention can start early
            proj_qk(0)
            for sb in range(SB):
                proj_v(sb)
            for cb in range(1, CB):
                proj_qk(cb)

        # ================= Stage B + C =================
        with ExitStack() as bctx:
            pb = bctx.enter_context(tc.tile_pool(name="pP", bufs=7))
            sdp = bctx.enter_context(tc.tile_pool(name="sdp", bufs=4))
            rp = bctx.enter_context(tc.tile_pool(name="rp", bufs=16))
            psS = bctx.enter_context(tc.tile_pool(name="psS", bufs=2, space="PSUM"))
            psPV = bctx.enter_context(tc.tile_pool(name="psPV", bufs=1, space="PSUM"))
            cp = bctx.enter_context(tc.tile_pool(name="stageC", bufs=2))
            zp = bctx.enter_context(tc.tile_pool(name="zP", bufs=3))
            od_p = bctx.enter_context(tc.tile_pool(name="odP", bufs=1))
            # od[tb] = [osa (768) | dsa (768)]
            od = [od_p.tile([128, 2 * C], F32, tag=f"od{i}", name=f"od{i}")
                  for i in range(SB)]

            for h in range(H):
                hb, hoff = h // 2, (h % 2) * 64
                ppvs = []
                pOs, pDs = [], []
                sds = []

                def emit_dsa_pv(psb):
                    pD = pb.tile([128, T], BF16, tag="pD", name="pD", bufs=6)
                    nc.scalar.activation(pD[:], sds[psb][:], AF.Exp)
                    pDs.append(pD)
                    if debug and h == 0 and psb == 0:
                        nc.sync.dma_start(dbg["dbg_pD"][:], pD[:])
                    for tbp in range(4):
                        ppv = ppvs[tbp]
                        for half in range(2):
                            tb = tbp * 2 + half
                            base = half * 130
                            nc.tensor.matmul(
                                ppv[:, base + 65:base + 130],
                                pD[:, tb * 128:(tb + 1) * 128],
                                vaug[psb][:, h * 65:h * 65 + 65],
                                start=False, stop=(psb == SB - 1))

                for sb in range(SB):
                    ps = psS.tile([128, T], F32, tag="psS", name="psS")
                    for tch in range(2):
                        nc.tensor.matmul(
                            ps[:, tch * 512:(tch + 1) * 512],
                            kT[hb][hoff:hoff + 64, sb * 128:(sb + 1) * 128],
                            qT[hb][hoff:hoff + 64, tch * 512:(tch + 1) * 512],
                            start=True, stop=True)
                    pO = pb.tile([128, T], BF16, tag="pO", name="pO", bufs=8)
                    nc.scalar.activation(pO[:], ps[:], AF.Exp)
                    pOs.append(pO)
                    sd = sdp.tile([128, T], F32, tag="sd", name="sd")
                    nc.vector.tensor_mul(sd[:], ps[:], dep_t[sb][:])
                    sds.append(sd)
                    if debug and h == 0 and sb == 0:
                        nc.sync.dma_start(dbg["dbg_pO"][:], pO[:])

                    # osa-PV for this s-block into the 4 tb-pair psum tiles
                    for tbp in range(4):
                        if sb == 0:
                            ppv = psPV.tile([128, 260], F32, tag=f"ppv{tbp}",
                                            name=f"ppv{tbp}")
                            ppvs.append(ppv)
                        ppv = ppvs[tbp]
                        for half in range(2):
                            tb = tbp * 2 + half
                            base = half * 130
                            # start=True clears has_written for the WHOLE
                            # bank: only the very first matmul into this psum
                            # tile may set it.
                            nc.tensor.matmul(
                                ppv[:, base:base + 65],
                                pO[:, tb * 128:(tb + 1) * 128],
                                vaug[sb][:, h * 65:h * 65 + 65],
                                start=(sb == 0 and half == 0),
                                stop=(sb == SB - 1))
                    # skewed dep branch: exp2 + dsa-PV for the previous block
                    if sb >= 1:
                        emit_dsa_pv(sb - 1)
                emit_dsa_pv(SB - 1)

                # normalize + evict into od[tb] = [osa | dsa]
                for tbp in range(4):
                    ppv = ppvs[tbp]
                    rec = rp.tile([128, 4], F32, tag="rec", name="rec")
                    den3 = ppv[:].rearrange("p (g d) -> p g d", d=65)[:, :, 64:65]
                    nc.vector.reciprocal(
                        rec[:].rearrange("p (g d) -> p g d", d=1), den3)
                    for half in range(2):
                        tb = tbp * 2 + half
                        base = half * 130
                        if BCAST_NORM:
                            out3 = od[tb][:].rearrange(
                                "p (b c) -> p b c", b=2)[:, :, h * 64:(h + 1) * 64]
                            in03 = ppv[:, base:base + 130].rearrange(
                                "p (b c) -> p b c", b=2)[:, :, 0:64]
                            recs = rec[:, 2 * half:2 * half + 2]
                            rec3 = bass.AP(tensor=recs.tensor, offset=recs.offset,
                                           ap=[*recs.ap, [0, 64]])
                            nc.vector.tensor_mul(out3, in03, rec3)
                        else:
                            nc.vector.tensor_scalar(
                                od[tb][:, h * 64:(h + 1) * 64],
                                ppv[:, base:base + 64],
                                rec[:, 2 * half:2 * half + 1], None, ALU.mult)
                            nc.vector.tensor_scalar(
                                od[tb][:, C + h * 64:C + (h + 1) * 64],
                                ppv[:, base + 65:base + 129],
                                rec[:, 2 * half + 1:2 * half + 2], None, ALU.mult)

            if debug:
                nc.sync.dma_start(dbg["dbg_qT0"][:], qT[0][:].bitcast(F32))
                nc.sync.dma_start(dbg["dbg_kT0"][:], kT[0][:].bitcast(F32))
                nc.sync.dma_start(dbg["dbg_vaug0"][:], vaug[0][:])
                nc.sync.dma_start(dbg["dbg_osa0"][:], od[0][:, 0:C])
                nc.sync.dma_start(dbg["dbg_dsa0"][:], od[0][:, C:2 * C])

            # ================= Stage C: gate + blend =================
            for tb in range(SB):
                tod = cp.tile([128, 2 * C], F32, tag="tod", name="tod")
                nc.scalar.activation(tod[:], od[tb][:], AF.Tanh)
                scr = cp.tile([128, 2 * C], F32, tag="scr", name="scr", bufs=1)
                z = zp.tile([128, 1], F32, tag="z", name="z")
                nc.vector.scalar_tensor_tensor(
                    out=scr[:], in0=tod[:], scalar=0.0, in1=wgb[:],
                    op0=ALU.bypass, op1=ALU.mult, accum_out=z[:])
                e = zp.tile([128, 1], F32, tag="e", name="e")
                # e = exp(-(z + bg)) = exp(-z + nbg)
                nc.scalar.activation(e[:], z[:], AF.Exp, bias=nbg_t[:], scale=-1.0)
                g = zp.tile([128, 1], F32, tag="g", name="g")
                nc.vector.tensor_scalar_add(e[:], e[:], 1.0)
                nc.vector.reciprocal(g[:], e[:])
                diff = cp.tile([128, C], F32, tag="diff", name="diff", bufs=1)
                nc.vector.tensor_sub(diff[:], od[tb][:, 0:C], od[tb][:, C:2 * C])
                outt = cp.tile([128, C], F32, tag="outt", name="outt")
                nc.vector.scalar_tensor_tensor(
                    out=outt[:], in0=diff[:], scalar=g[:], in1=od[tb][:, C:2 * C],
                    op0=ALU.mult, op1=ALU.add)
                nc.sync.dma_start(out_d[tb * 128:(tb + 1) * 128, :], outt[:])

    nc.finalize()
    return nc


_CACHE = {}


def _prep_in_maps(hidden_states, dependency_matrix, Wq, bq, Wk, bk, Wv, bv, Wg, bg):
    hs = np.asarray(hidden_states, dtype=np.float32)
    dep = np.asarray(dependency_matrix, dtype=np.float32)
    shared = {
        "wq": np.ascontiguousarray(np.asarray(Wq, np.float32).T),
        "wk": np.ascontiguousarray(np.asarray(Wk, np.float32).T),
        "wv": np.ascontiguousarray(np.asarray(Wv, np.float32).T),
        "bqs": (np.asarray(bq, np.float32) * 0.125).reshape(C, 1),
        "bk": np.asarray(bk, np.float32).reshape(C, 1),
        "bv": np.ascontiguousarray(np.asarray(bv, np.float32).reshape(C)),
        "wg": np.ascontiguousarray(np.asarray(Wg, np.float32).reshape(2 * C)),
        "nbg": (-np.asarray(bg, np.float32)).reshape(1),
    }
    in_maps = []
    for b in range(B):
        m = dict(shared)
        m["xT"] = np.ascontiguousarray(hs[b].T)
        m["dep"] = np.ascontiguousarray(dep[b].T).astype(ml_dtypes.bfloat16)
        in_maps.append(m)
    return in_maps


def kernel(**inputs):
    from concourse.bass_utils import run_bass_kernel_spmd
    if "nc" not in _CACHE:
        _CACHE["nc"] = build_nc()
    nc = _CACHE["nc"]
    in_maps = _prep_in_maps(**inputs)
    res = run_bass_kernel_spmd(nc, in_maps, core_ids=list(range(NCORES)))
    out = np.stack([res.results[i]["out"] for i in range(NCORES)], axis=0)
    return out.astype(np.float32)
